# revision 13
# baseline (speedup 1.0000x reference)
"""MoH (mixture-of-heads) attention kernel for 8 Trainium2 NeuronCores.

Problem (hardcoded shapes): x [2, 2048, 1024], 16 heads x 64 dim.
  q,k,v = x @ W{q,k,v}.T + b      -> [B, H, N, hd]
  q     = q / ||q||; q = (q + query_embedding) * softplus(temperature)
  h     = softmax(q k^T / sqrt(hd)) v  -> [B, N, 1024]
  gates = softmax(h @ Wr.T + br); top-3 mask; sw = softmax(h @ Ws.T + bs)
  g     = 2*sw0 + 6*sw1*sum(top3(gates))      (per-token scalar)
  out   = (h * g) @ Wp.T + bp

Sharding: token-parallel. Core c (of 8) owns batch b=c//4 and its token
block [512*(c%4), 512*(c%4)+512).  Each core projects q/k/v for its own
512 tokens; k (fp8) and v (bf16) shards are AllGathered within the 4-core
group of the same batch; attention (512 queries x 2048 keys), routing
gates and the output projection then run locally.

Numerics (emulated end-to-end max scale-rel err ~1.5e-2 vs the 2e-2 gate):
  * q/k projections run DoubleRow fp8 (x, Wq, Wk in fp8e4; weights
    pre-scaled x8 on host to dodge fp8 subnormals, 1/8 folded into the
    psum->sbuf casts).  k stays fp8 through the AllGather and as the
    zero-padded qk stationary.  v/h/Wp/et stay bf16 (fp8 there measured
    3-5e-2 end-to-end - too hot).
  * v carries bv (softmax rows sum to 1 => attn@(v+bv) == h+bv), so the
    per-head bias tail disappears.
  * exp() is split ScalarE/DVE: half the tiles use the ACT table exp, half
    use a Schraudolph-style DVE op - uint16(184.665*x + 16249) bitcast to
    bf16 is e^x to ~1.8% rms, one tensor_scalar per tile.
  * softmax denominators: per 4-head group one [4,512]
    reciprocal_approx_fast + 4 tiny [4,64]-stationary broadcast matmuls
    (replaces 53us of serial single-partition RECIPROCALs).

Schedule:
  * weight/const DMAs at t=0 on the sync queue; the scalar queue is kept
    free so the fp8 k payload cast (ACT Copy) runs as soon as the k psum
    lands; gpsimd is reserved for collective payload DMAs + triggers, so
    AllGather g triggers as soon as group g's payload is cast (~7us for
    g0).  The first collective still can't execute before the ~64us
    global-comm init/launch-skew floor - that is environmental.
  * per-group tails: batched reciprocal + division into packed bf16 hT16,
    then partial routing-gate and output-projection contractions for the
    two finished s-blocks accumulate into SBUF (prs_acc / oacc), leaving
    only a ~14us tail after the last head.
"""

import numpy as np
from contextlib import ExitStack

import concourse.bacc as bacc
import concourse.bass as bass
import concourse.tile as tile
from concourse import mybir
from concourse.bass_utils import run_bass_kernel_spmd
import ml_dtypes

BF16NP = ml_dtypes.bfloat16
F8NP = ml_dtypes.float8_e4m3

F32 = mybir.dt.float32
F32R = mybir.dt.float32r
BF16 = mybir.dt.bfloat16
F8 = mybir.dt.float8e4
U16 = mybir.dt.uint16
AF = mybir.ActivationFunctionType
ALU = mybir.AluOpType
AX = mybir.AxisListType
DR = mybir.MatmulPerfMode.DoubleRow

B, N, D = 2, 2048, 1024
H, HD = 16, 64
NCORE = 8
TOK = 512                      # tokens per core
KT = N // 128                  # 16 k-token tiles per batch
GROUPS = [[0, 1, 2, 3], [4, 5, 6, 7]]
GSZ = 2 * 128 * TOK            # one group's k (or v) payload, flat elems
CCB = 3 * GSZ                  # payload bytes/core/group: k fp8 + v bf16
WSCALE = 8.0                   # host premultiplies Wq/Wk by this (fp8)

# which of the 8 per-head exp tiles run on DVE (Schraudolph) vs ScalarE
DVE_EXP = (1, 3, 5, 7)
SCH_A = 128 * 1.4426950408889634   # uint16 bf16-bits = SCH_A*x + SCH_B
SCH_B = 16249.0


def build_nc():
    nc = bacc.Bacc(None, target_bir_lowering=False, num_devices=NCORE)

    xT8 = nc.declare_dram_parameter("xT8", [128, 8, TOK], F8, isOutput=False)
    xT16 = nc.declare_dram_parameter("xT16", [128, 8, TOK], BF16, isOutput=False)
    wqT = nc.declare_dram_parameter("wqT", [4, 128, 8, 256], F8, isOutput=False)
    wkT = nc.declare_dram_parameter("wkT", [4, 128, 8, 256], F8, isOutput=False)
    wvT = nc.declare_dram_parameter("wvT", [4, 128, 8, 256], BF16, isOutput=False)
    wpT = nc.declare_dram_parameter("wpT", [128, 8, D], BF16, isOutput=False)
    wrsT = nc.declare_dram_parameter("wrsT", [D, 17], F32R, isOutput=False)
    bq = nc.declare_dram_parameter("bq", [D], F32, isOutput=False)
    bv = nc.declare_dram_parameter("bv", [D], F32, isOutput=False)
    bp = nc.declare_dram_parameter("bp", [D], F32, isOutput=False)
    brs = nc.declare_dram_parameter("brs", [17], F32, isOutput=False)
    temp16 = nc.declare_dram_parameter("temp16", [16], F32, isOutput=False)
    qe = nc.declare_dram_parameter("qe", [H, HD], F32, isOutput=False)
    msel = nc.declare_dram_parameter("msel", [8, 128, 16], BF16, isOutput=False)
    esel = nc.declare_dram_parameter("esel", [8, 16, 128], F32R, isOutput=False)
    esel4 = nc.declare_dram_parameter("esel4", [4, 4, HD], F32R, isOutput=False)
    ident = nc.declare_dram_parameter("ident", [128, 128], F32, isOutput=False)
    out = nc.declare_dram_parameter("out", [TOK, D], F32, isOutput=True)

    with tile.TileContext(nc) as tc, ExitStack() as ctx:
        const = ctx.enter_context(tc.tile_pool(name="const", bufs=1))
        psum = ctx.enter_context(tc.tile_pool(name="psum", bufs=2, space="PSUM"))
        dram = ctx.enter_context(tc.tile_pool(name="dram", bufs=1, space="DRAM"))
        wpool = ctx.enter_context(tc.tile_pool(name="wpool", bufs=1))
        work = ctx.enter_context(tc.tile_pool(name="work", bufs=1))
        kv = ctx.enter_context(tc.tile_pool(name="kv", bufs=2))

        # ---- t=0 bulk DMAs (sync queue; scalar stays free for kc casts) --
        wk_sb = [wpool.tile([128, 8, 256], F8, name=f"wk_sb{g}") for g in range(4)]
        wv_sb = [wpool.tile([128, 8, 256], BF16, name=f"wv_sb{g}") for g in range(4)]
        wq_sb = [wpool.tile([128, 8, 256], F8, name=f"wq_sb{g}") for g in range(4)]
        wp_sb = wpool.tile([128, 8, D], BF16)
        x8 = work.tile([128, 8, TOK], F8)
        x16 = work.tile([128, 8, TOK], BF16)

        nc.sync.dma_start(out=wk_sb[0], in_=wkT[0])
        nc.sync.dma_start(out=x8, in_=xT8[:, :, :])
        nc.sync.dma_start(out=x16, in_=xT16[:, :, :])
        nc.sync.dma_start(out=wv_sb[0], in_=wvT[0])
        bv_rep = const.tile([128, 4, 256], F32)
        nc.sync.dma_start(
            out=bv_rep,
            in_=bv.rearrange("(g o) -> g o", g=4)[None, :, :].to_broadcast(
                [128, 4, 256]))
        nc.sync.dma_start(out=wk_sb[1], in_=wkT[1])
        nc.sync.dma_start(out=wv_sb[1], in_=wvT[1])
        bq_ch = const.tile([128, 8], F32)
        nc.sync.dma_start(out=bq_ch, in_=bq.rearrange("(s p) -> p s", p=128))
        for g in range(4):
            nc.sync.dma_start(out=wq_sb[g], in_=wqT[g])
        temp_sb = const.tile([16, 1], F32)
        nc.sync.dma_start(out=temp_sb, in_=temp16[:, None])
        qe_sb = const.tile([16, HD], F32)
        nc.sync.dma_start(out=qe_sb, in_=qe[:, :])
        msel_sb = const.tile([128, 8, 16], BF16)
        nc.sync.dma_start(out=msel_sb, in_=msel.rearrange("s p h -> p s h"))
        esel_sb = const.tile([16, 8, 128], F32R)
        nc.sync.dma_start(out=esel_sb, in_=esel.rearrange("s h m -> h s m"))
        esel4_sb = const.tile([4, 4, HD], F32R)
        nc.sync.dma_start(out=esel4_sb, in_=esel4[:, :, :])
        nc.sync.dma_start(out=wk_sb[2], in_=wkT[2])
        nc.sync.dma_start(out=wv_sb[2], in_=wvT[2])
        nc.sync.dma_start(out=wk_sb[3], in_=wkT[3])
        nc.sync.dma_start(out=wv_sb[3], in_=wvT[3])
        brs_sb = const.tile([17, 1], F32)
        nc.sync.dma_start(out=brs_sb, in_=brs[:, None])
        ident_sb = const.tile([128, 128], F32)
        nc.sync.dma_start(out=ident_sb, in_=ident[:, :])
        w_rs = const.tile([128, 8, 17], F32R)
        nc.sync.dma_start(out=w_rs, in_=wrsT.rearrange("(s p) o -> p s o", p=128))
        nc.sync.dma_start(out=wp_sb, in_=wpT[:, :, :])
        bp_rep = const.tile([128, D], F32)
        nc.sync.dma_start(out=bp_rep, in_=bp[None, :].to_broadcast([128, D]))

        # persistent zero-padded fp8 k stationaries (even heads: data in
        # partitions 0-63, odd heads: 64-127; pad memset ONCE)
        kt8 = [work.tile([128, 4, TOK], F8, name=f"kt8_{i}") for i in range(4)]
        for i in range(4):
            if i < 2:
                nc.vector.memset(kt8[i][64:128, :, :], 0.0)
            else:
                nc.vector.memset(kt8[i][0:64, :, :], 0.0)
        # persistent bf16 v stationaries with the ones (denominator) column
        vts = [work.tile([128, KT, HD + 1], BF16, name=f"vt{i}") for i in range(3)]
        for i in range(3):
            nc.vector.memset(vts[i][:, :, HD:HD + 1], 1.0)

        cc_in = [dram.tile([CCB], F8, name=f"ccin{g}") for g in range(4)]
        cc_out = [dram.tile([4 * CCB], F8, name=f"ccout{g}") for g in range(4)]

        # ---- stage 1: projections + 4 pipelined AllGathers --------------
        for g in range(4):
            cin_k = cc_in[g][0:GSZ].rearrange("(s p t) -> p s t", p=128, t=TOK)
            cin_v = cc_in[g][GSZ:CCB].bitcast(BF16).rearrange(
                "(m p o) -> p m o", p=128, o=256)
            # k projection: DoubleRow fp8, contraction pairs = consecutive
            # 128-channel blocks
            pk = psum.tile([128, 2, TOK], F32, tag="mm", bufs=3, name=f"pk{g}")
            for k4 in range(4):
                for half in range(2):
                    nc.tensor.matmul(
                        pk[:, half, :],
                        wk_sb[g][:, 2 * k4:2 * k4 + 2, 128 * half:128 * half + 128],
                        x8[:, 2 * k4:2 * k4 + 2, :],
                        start=(k4 == 0), stop=(k4 == 3), perf_mode=DR)
            # NOTE: k bias dropped -- q.(k+bk) differs from q.k by a
            # per-query constant, which cancels in the softmax exactly
            kc = kv.tile([128, 2, TOK], F8, tag="kc", name=f"kc{g}")
            nc.scalar.activation(kc, pk, AF.Copy, scale=1.0 / WSCALE)
            nc.gpsimd.dma_start(out=cin_k, in_=kc)

            pvt = [psum.tile([128, 2, TOK], F32, tag="mm", bufs=3,
                             name=f"pvt{g}_{i}") for i in range(2)]
            for ks in range(8):
                for mt in range(4):
                    nc.tensor.matmul(pvt[mt // 2][:, mt % 2, 0:256],
                                     x16[:, ks, 128 * mt:128 * mt + 128],
                                     wv_sb[g][:, ks, :], start=(ks == 0),
                                     stop=(ks == 7))
            # v' = v + bv folded here (softmax rows sum to 1, so
            # attn @ (v+bv) == h + bv -- no per-head bias tail needed)
            for i in range(2):
                vc = kv.tile([128, 2, 256], BF16, tag="vc", name=f"vc{g}_{i}")
                nc.vector.tensor_tensor(
                    vc, pvt[i][:, :, 0:256],
                    bv_rep[:, g, None, 0:256].to_broadcast([128, 2, 256]),
                    ALU.add)
                nc.gpsimd.dma_start(out=cin_v[:, 2 * i:2 * i + 2, :], in_=vc)

            nc.gpsimd.collective_compute(
                "AllGather", ALU.bypass, replica_groups=GROUPS,
                ins=[cc_in[g].opt()], outs=[cc_out[g].opt()])

            if g == 0:
                # q projection + q-norm while AllGather 0 is in flight
                # softplus(t) = ln(1 + exp(t)) -- keeps ACT on one table set
                sp8 = const.tile([16, 1], F32)
                nc.scalar.activation(sp8, temp_sb, AF.Exp)
                nc.vector.tensor_scalar_add(sp8, sp8, 1.0)
                nc.scalar.activation(sp8, sp8, AF.Ln)
                nc.vector.tensor_scalar_mul(sp8, sp8, 0.125)
                qe_sp16 = const.tile([16, HD], F32)
                nc.vector.tensor_tensor(qe_sp16, qe_sb,
                                        sp8[:, 0:1].to_broadcast([16, HD]), ALU.mult)
                # reshape [16,64](head-major) -> [128,8](channel-major) via DRAM
                qe_scr = dram.tile([D], F32)
                nc.sync.dma_start(out=qe_scr.rearrange("(h d) -> h d", h=16), in_=qe_sp16)
                qe_ch = const.tile([128, 8], F32)
                nc.sync.dma_start(out=qe_ch, in_=qe_scr.rearrange("(s p) -> p s", p=128))

                q_sb = work.tile([128, 8, TOK], BF16)
                for sp_ in range(4):
                    pq = psum.tile([128, 2, TOK], F32, tag="mm", bufs=3,
                                   name=f"pq{sp_}")
                    for k4 in range(4):
                        for half in range(2):
                            nc.tensor.matmul(
                                pq[:, half, :],
                                wq_sb[sp_][:, 2 * k4:2 * k4 + 2,
                                           128 * half:128 * half + 128],
                                x8[:, 2 * k4:2 * k4 + 2, :],
                                start=(k4 == 0), stop=(k4 == 3), perf_mode=DR)
                    nc.vector.scalar_tensor_tensor(
                        q_sb[:, 2 * sp_:2 * sp_ + 2, :], pq, 1.0 / WSCALE,
                        bq_ch[:, 2 * sp_:2 * sp_ + 2, None].to_broadcast(
                            [128, 2, TOK]), ALU.mult, ALU.add)

                # q-norm + scale + query-embedding, all channel-major
                pss = psum.tile([16, TOK], F32, tag="pv", bufs=2)
                for s in range(8):
                    sq_t = kv.tile([128, TOK], BF16, tag="sq", name=f"sq{s}")
                    nc.vector.tensor_mul(sq_t, q_sb[:, s, :], q_sb[:, s, :])
                    nc.tensor.matmul(pss, msel_sb[:, s, :], sq_t,
                                     start=(s == 0), stop=(s == 7))
                # rsqrt(ss) = exp(-0.5 * ln(ss)) -- same exp/ln ACT table set
                sqs = const.tile([16, TOK], F32)
                nc.scalar.activation(sqs, pss, AF.Ln)
                rr = const.tile([16, TOK], F32)
                nc.scalar.activation(rr, sqs, AF.Exp, scale=-0.5)
                rs_sp = const.tile([16, TOK], F32R)
                nc.vector.tensor_tensor(rs_sp, rr, sp8[:, 0:1].to_broadcast([16, TOK]),
                                        ALU.mult)
                qs_m = work.tile([128, 8, TOK], BF16)
                for s in range(8):
                    pb = psum.tile([128, TOK], F32, tag="pv", bufs=2, name=f"pb{s}")
                    nc.tensor.matmul(pb, esel_sb[:, s, :], rs_sp, start=True, stop=True)
                    nc.vector.tensor_mul(q_sb[:, s, :], q_sb[:, s, :], pb)
                    nc.vector.tensor_tensor(
                        qs_m[:, s, :], q_sb[:, s, :],
                        qe_ch[:, s:s + 1].to_broadcast([128, TOK]), ALU.add)

        # ---- stage 2: attention (group g overlaps AllGather g+1) --------
        w_rs16 = const.tile([128, 8, 17], BF16)
        nc.vector.tensor_copy(w_rs16, w_rs)
        hT = work.tile([HD + 1, H, TOK], F32)     # per-head num (0-63) + den (64)
        hT16 = work.tile([128, 8, TOK], BF16)     # packed channel-major h
        prs_acc = work.tile([17, TOK], F32)       # routing logits accumulator
        oacc = work.tile([128, 2, 4, TOK], F32)   # [tok, nt, mt, col] out acc
        for h in range(H):
            g = h // 4
            s = h // 2
            odd = h % 2
            v_h = vts[h % 3]
            for j in range(4):
                vj = cc_out[g][j * CCB + GSZ:(j + 1) * CCB].bitcast(BF16).rearrange(
                    "(m p o) -> p m o", p=128, o=256)
                nc.gpsimd.dma_start(
                    out=v_h[:, 4 * j:4 * j + 4, 0:HD],
                    in_=vj[:, :, HD * (h % 4):HD * (h % 4) + HD])

            ktile = kt8[2 * odd + (h // 2) % 2]
            kall = cc_out[g].rearrange("(j c) -> j c", j=4)[
                :, (s % 2) * 128 * TOK:((s % 2) + 1) * 128 * TOK].rearrange(
                "j (p t) -> p j t", p=128)
            if odd:
                nc.sync.dma_start(out=ktile[64:128, :, :], in_=kall[64:128, :, :])
            else:
                nc.sync.dma_start(out=ktile[0:64, :, :], in_=kall[0:64, :, :])

            ppv = psum.tile([HD + 1, TOK], F32, tag="pv", bufs=2, name=f"ppv{h}")
            for ktg in range(8):
                ps_ = psum.tile([128, 2, TOK], F32, tag="mm", bufs=3,
                                name=f"ps{h}_{ktg}")
                for half in range(2):
                    kt = 2 * ktg + half
                    nc.tensor.matmul(
                        ps_[:, half, :],
                        ktile[:, kt // 4, 128 * (kt % 4):128 * (kt % 4) + 128],
                        qs_m[:, s, :], start=True, stop=True)
                et = kv.tile([128, 2, TOK], BF16, tag="et", bufs=5,
                             name=f"et{h}_{ktg}")
                if ktg in DVE_EXP:
                    # Schraudolph in bf16-bit space: uint16(184.665x + 16249)
                    # viewed as bf16 is e^x to ~1.8% rms; qk logits are in
                    # [-4, 4] so the bits stay far from wrap/saturation
                    with nc.allow_low_precision(reason="schraudolph exp"):
                        nc.vector.tensor_scalar(
                            et.bitcast(U16), ps_, SCH_A, SCH_B, ALU.mult, ALU.add)
                else:
                    nc.scalar.activation(et, ps_, AF.Exp)
                for half in range(2):
                    kt = 2 * ktg + half
                    nc.tensor.matmul(ppv, v_h[:, kt, :], et[:, half, :],
                                     start=(kt == 0), stop=(kt == KT - 1))
            nc.vector.tensor_copy(hT[:, h, :], ppv)

            if h % 4 == 3:
                # batched denominators for the group: gather the 4 den rows,
                # one fast reciprocal, 4 tiny broadcast matmuls, then the
                # h/den division straight into the packed bf16 hT16
                den4 = kv.tile([4, TOK], F32, tag="den", name=f"den{g}")
                for hh in range(4):
                    nc.sync.dma_start(out=den4[hh:hh + 1, :],
                                      in_=hT[HD:HD + 1, h - 3 + hh, :])
                rc4 = kv.tile([4, TOK], F32, tag="rc", name=f"rc{g}")
                nc.vector.reciprocal_approx_fast(out=rc4, in_=den4)
                rc4r = kv.tile([4, TOK], F32R, tag="rcr", name=f"rcr{g}")
                with nc.allow_low_precision(reason="f32r recip broadcast"):
                    nc.vector.tensor_copy(rc4r, rc4)
                for hh in range(h - 3, h + 1):
                    s2 = hh // 2
                    pg = psum.tile([HD, TOK], F32, tag="pv", bufs=2, name=f"pg{hh}")
                    nc.tensor.matmul(pg, esel4_sb[:, hh % 4, :], rc4r,
                                     start=True, stop=True)
                    if hh % 2:
                        tod = kv.tile([HD, TOK], BF16, tag="tod", name=f"tod{hh}")
                        nc.vector.tensor_mul(tod, hT[0:HD, hh, :], pg)
                        nc.sync.dma_start(out=hT16[64:128, s2, :], in_=tod)
                    else:
                        nc.vector.tensor_mul(hT16[0:64, s2, :],
                                             hT[0:HD, hh, :], pg)

                # partial routing-logit + output-projection contractions for
                # the two finished s-blocks, accumulated in SBUF
                prs = psum.tile([17, TOK], F32, tag="pv", bufs=2, name=f"prs{g}")
                for s2 in (2 * g, 2 * g + 1):
                    nc.tensor.matmul(prs, w_rs16[:, s2, :], hT16[:, s2, :],
                                     start=(s2 == 2 * g), stop=(s2 == 2 * g + 1))
                if g == 0:
                    nc.vector.tensor_copy(prs_acc, prs)
                else:
                    nc.vector.tensor_add(prs_acc, prs_acc, prs)
                for nt in range(2):
                    po = [psum.tile([128, 2, TOK], F32, tag="mm", bufs=3,
                                    name=f"po{g}_{nt}_{i}") for i in range(2)]
                    for s2 in (2 * g, 2 * g + 1):
                        for mt in range(4):
                            nc.tensor.matmul(
                                po[mt // 2][:, mt % 2, :],
                                hT16[:, s2, 128 * mt:128 * mt + 128],
                                wp_sb[:, s2, TOK * nt:TOK * nt + TOK],
                                start=(s2 == 2 * g), stop=(s2 == 2 * g + 1))
                    for i in range(2):
                        if g == 0:
                            nc.vector.tensor_copy(oacc[:, nt, 2 * i:2 * i + 2, :],
                                                  po[i])
                        else:
                            nc.vector.tensor_add(oacc[:, nt, 2 * i:2 * i + 2, :],
                                                 oacc[:, nt, 2 * i:2 * i + 2, :],
                                                 po[i])

        # ---- stage 3: routing gates -> per-token scalar g ---------------
        rs_sb = const.tile([17, TOK], F32)
        nc.vector.tensor_tensor(rs_sb, prs_acc,
                                brs_sb[:, 0:1].to_broadcast([17, TOK]), ALU.add)
        lg_t = const.tile([128, 4, 17], F32)
        for c4 in range(4):
            pt_ = psum.tile([128, 17], F32, tag="pv", bufs=2, name=f"pt{c4}")
            nc.tensor.transpose(pt_, rs_sb[:, 128 * c4:128 * c4 + 128],
                                ident_sb[0:17, 0:17])
            nc.vector.tensor_copy(lg_t[:, c4, :], pt_)

        e15 = const.tile([128, 4, 15], F32)
        nc.scalar.activation(e15, lg_t[:, :, 0:15], AF.Exp)
        e2 = const.tile([128, 4, 2], F32)
        nc.scalar.activation(e2, lg_t[:, :, 15:17], AF.Exp)
        s15 = const.tile([128, 4, 1], F32)
        nc.vector.tensor_reduce(s15, e15, AX.X, ALU.add)
        s2 = const.tile([128, 4, 1], F32)
        nc.vector.tensor_reduce(s2, e2, AX.X, ALU.add)
        m1 = const.tile([128, 4, 1], F32)
        nc.vector.tensor_reduce(m1, e15, AX.X, ALU.max)
        msk = const.tile([128, 4, 15], F32)
        nc.vector.tensor_tensor(msk, e15, m1.to_broadcast([128, 4, 15]), ALU.is_ge)
        e15b = const.tile([128, 4, 15], F32)
        nc.vector.scalar_tensor_tensor(e15b, msk, -1e30, e15, ALU.mult, ALU.add)
        m2 = const.tile([128, 4, 1], F32)
        nc.vector.tensor_reduce(m2, e15b, AX.X, ALU.max)
        nc.vector.tensor_tensor(msk, e15b, m2.to_broadcast([128, 4, 15]), ALU.is_ge)
        nc.vector.scalar_tensor_tensor(e15b, msk, -1e30, e15b, ALU.mult, ALU.add)
        m3 = const.tile([128, 4, 1], F32)
        nc.vector.tensor_reduce(m3, e15b, AX.X, ALU.max)
        nc.vector.tensor_add(m1, m1, m2)
        nc.vector.tensor_add(m1, m1, m3)       # m1 = top3 sum of e15
        nc.vector.reciprocal(s15, s15)
        nc.vector.reciprocal(s2, s2)
        ga = const.tile([128, 4, 1], F32)
        nc.vector.tensor_mul(ga, e2[:, :, 0:1], s2)
        gb = const.tile([128, 4, 1], F32)
        nc.vector.tensor_mul(gb, e2[:, :, 1:2], s2)
        nc.vector.tensor_mul(gb, gb, m1)
        nc.vector.tensor_mul(gb, gb, s15)
        nc.vector.tensor_scalar_mul(gb, gb, 6.0)
        gg = const.tile([128, 4, 1], F32)
        nc.vector.scalar_tensor_tensor(gg, ga, 2.0, gb, ALU.mult, ALU.add)

        # ---- stage 4: scale accumulated projection, add bias, store -----
        for nt in range(2):
            for mt in range(4):
                ob = kv.tile([128, TOK], F32, tag="ob", bufs=2,
                             name=f"ob{nt}_{mt}")
                nc.vector.tensor_mul(ob, oacc[:, nt, mt, :],
                                     gg[:, mt, 0:1].to_broadcast([128, TOK]))
                nc.vector.tensor_add(ob, ob, bp_rep[:, TOK * nt:TOK * nt + TOK])
                nc.sync.dma_start(
                    out=out[128 * mt:128 * mt + 128, TOK * nt:TOK * nt + TOK],
                    in_=ob)

    nc.compile()
    return nc


_NC_CACHE = {}


def _get_nc():
    if "nc" not in _NC_CACHE:
        _NC_CACHE["nc"] = build_nc()
    return _NC_CACHE["nc"]


def _wdev(W, scale, npdt):
    """[out,in] weight -> device layout [g, p, ks, c]."""
    wT = (np.asarray(W, np.float32).T * scale).astype(npdt)   # [in, out]
    return np.ascontiguousarray(
        wT.reshape(8, 128, 4, 256).transpose(2, 1, 0, 3))


def _host_prep(x, Wq, bq, Wk, bk, Wv, bv, Wp, bp, Wr, br, Ws, bs,
               temperature, query_embedding):
    f32 = np.float32
    xf = np.ascontiguousarray(x, dtype=f32).reshape(B * N, D)
    shared = {
        "wqT": _wdev(Wq, WSCALE, F8NP), "wkT": _wdev(Wk, WSCALE, F8NP),
        "wvT": _wdev(Wv, 1.0, BF16NP),
        "wpT": np.ascontiguousarray(
            np.asarray(Wp, f32).T.astype(BF16NP).reshape(
                8, 128, D).transpose(1, 0, 2)),
        "wrsT": np.ascontiguousarray(
            np.concatenate([np.asarray(Wr, f32), np.asarray(Ws, f32)], 0).T),
        "bq": np.ascontiguousarray(bq, f32),
        "bv": np.ascontiguousarray(bv, f32), "bp": np.ascontiguousarray(bp, f32),
        "brs": np.concatenate([np.asarray(br, f32), np.asarray(bs, f32)]),
        "temp16": np.ascontiguousarray(np.asarray(temperature, f32).reshape(H)),
        "qe": np.ascontiguousarray(np.asarray(query_embedding, f32).reshape(H, HD)),
        "ident": np.eye(128, dtype=f32),
    }
    ch = np.arange(D)
    head_of_ch = ch // HD
    msel = np.zeros((8, 128, 16), BF16NP)
    esel = np.zeros((8, 16, 128), f32)
    for s in range(8):
        hh = head_of_ch[128 * s:128 * s + 128]
        msel[s, np.arange(128), hh] = 1.0
        esel[s, hh, np.arange(128)] = 1.0
    shared["msel"] = msel
    shared["esel"] = esel
    # esel4[:, hh, :]: [4, 64] selector - row hh ones (broadcasts head hh's
    # reciprocal-denominator row onto 64 partitions)
    e4 = np.zeros((4, 4, HD), f32)
    for hh in range(4):
        e4[hh, hh, :] = 1.0
    shared["esel4"] = e4

    in_maps = []
    for c in range(NCORE):
        rows = slice((c // 4) * N + TOK * (c % 4),
                     (c // 4) * N + TOK * (c % 4) + TOK)
        m = dict(shared)
        xTc = xf[rows].T.reshape(8, 128, TOK).transpose(1, 0, 2)
        m["xT8"] = np.ascontiguousarray(xTc.astype(F8NP))
        m["xT16"] = np.ascontiguousarray(xTc.astype(BF16NP))
        in_maps.append(m)
    return in_maps


def kernel(**inputs):
    nc = _get_nc()
    in_maps = _host_prep(**inputs)
    res = run_bass_kernel_spmd(nc, in_maps, core_ids=list(range(NCORE)))
    shards = [res.results[c]["out"] for c in range(NCORE)]
    return np.concatenate(shards, 0).reshape(B, N, D)
